# revision 5
# baseline (speedup 1.0000x reference)
"""Trainium2 Bass kernel for nn_MoELayer (dense MoE + attention-over-experts).

Strategy
--------
Data-parallel over batch across 8 NeuronCores (4096 rows/core), params
replicated. Host-side algebraic folding (exact math, done in float64):

    expert_keys   = h @ (W2 Ke) + b2 Ke        (h = relu(x W1 + b1))
    scores[b,e]   = h . G[e, task[b]] + c[e, task[b]],  G[e,t] = (W2 Ke) q_t
    expert_values = h @ (W2 Ve) + b2 Ve

so expert_outputs / keys never materialize. Per 512-column batch tile,
per expert, the device does:
  L1:   hT_e[256,512]  = W1[e]^T-matmul (PE, fp32r)   + relu/bias (ACT)
  VS:   [80,512] psum  = [W2V[e] | G[e]^T]-matmul: rows 0:64 = values^T,
        rows 64:80 = scores vs all 16 task queries
  mask: (psum + bias) * M  (DVE scalar_tensor_tensor), M rows 64:80 = onehot(task)
  sel:  basis matmul accumulates the task-selected score into S32[32,512]
Softmax runs in batch-partition layout via PE transposes; the combine
pre-broadcasts w (K=1 PE outer product), scales values (DVE) and
accumulates sum_e w_e * ev_e on PE into yT, transposed back for output.

float32r (fp32 rounded to 11 mantissa bits, RNE) runs matmuls at 4x the
fp32 rate; set MATMUL_DTYPE = "float32" for the exact-fp32 fallback.
"""

import os
import sys

for _p in ("/opt/trn_rl_repo", "/root/.axon_site/_ro/trn_rl_repo"):
    if os.path.isdir(_p) and _p not in sys.path:
        sys.path.insert(0, _p)

import numpy as np
from contextlib import ExitStack

import concourse.bacc as bacc
import concourse.mybir as mybir
import concourse.tile as tile
from concourse.bass_utils import run_bass_kernel_spmd

F32 = mybir.dt.float32
F32R = mybir.dt.float32r
AL = mybir.AluOpType
ACTF = mybir.ActivationFunctionType

B, D_IN, H1, D_OUT = 32768, 256, 256, 64
E, T, MU = 32, 16, 0.01
NCORES = 8
BC = B // NCORES          # 4096 rows per core
F = 512                   # batch columns per tile
NF = BC // F              # 8 tiles per core
EPS = 1e-6

MATMUL_DTYPE = os.environ.get("MOE_MM_DTYPE", "float32r")
RDT = F32R if MATMUL_DTYPE == "float32r" else F32


def _build_program():
    nc = bacc.Bacc("TRN2", target_bir_lowering=False, debug=False)

    x_d = nc.dram_tensor("x", [BC, D_IN], F32, kind="ExternalInput").ap()
    ut_d = nc.dram_tensor("ut", [T, BC], F32, kind="ExternalInput").ap()
    w1_d = nc.dram_tensor("w1", [128, E, 2, H1], RDT, kind="ExternalInput").ap()
    waug_d = nc.dram_tensor("waug", [128, E, 2, 80], RDT, kind="ExternalInput").ap()
    b1_d = nc.dram_tensor("b1r", [128, 2, E], F32, kind="ExternalInput").ap()
    b2aug_d = nc.dram_tensor("b2aug", [80, E], F32, kind="ExternalInput").ap()
    zc_d = nc.dram_tensor("zc", [80, 63], RDT, kind="ExternalInput").ap()
    i64_d = nc.dram_tensor("i64", [64, 64], RDT, kind="ExternalInput").ap()
    ones64_d = nc.dram_tensor("ones64", [1, 64], RDT, kind="ExternalInput").ap()
    ident_d = nc.dram_tensor("ident", [128, 128], F32, kind="ExternalInput").ap()
    onescol_d = nc.dram_tensor("onescol", [128, 1], F32, kind="ExternalInput").ap()

    y_d = nc.dram_tensor("y", [BC, D_OUT], F32, kind="ExternalOutput").ap()
    rs_d = nc.dram_tensor("rsum", [1, 1], F32, kind="ExternalOutput").ap()

    x_t = x_d.rearrange("(nf s p) i -> nf s p i", s=4, p=128)     # [NF,4,128,256]
    y_t = y_d.rearrange("(nf s p) j -> nf s p j", s=4, p=128)

    with tile.TileContext(nc) as tc, ExitStack() as ctx:
        consts = ctx.enter_context(tc.tile_pool(name="consts", bufs=1))
        evp = ctx.enter_context(tc.tile_pool(name="evp", bufs=1))
        xp = ctx.enter_context(tc.tile_pool(name="xp", bufs=2))
        xtp = ctx.enter_context(tc.tile_pool(name="xtp", bufs=2))
        hp = ctx.enter_context(tc.tile_pool(name="hp", bufs=3))
        smp = ctx.enter_context(tc.tile_pool(name="smp", bufs=2))
        outp = ctx.enter_context(tc.tile_pool(name="outp", bufs=2))

        pH = ctx.enter_context(tc.tile_pool(name="pH", bufs=2, space="PSUM"))
        pVS = ctx.enter_context(tc.tile_pool(name="pVS", bufs=2, space="PSUM"))
        pS32 = ctx.enter_context(tc.tile_pool(name="pS32", bufs=1, space="PSUM"))
        pT = ctx.enter_context(tc.tile_pool(name="pT", bufs=1, space="PSUM"))
        pB = ctx.enter_context(tc.tile_pool(name="pB", bufs=1, space="PSUM"))
        pY = ctx.enter_context(tc.tile_pool(name="pY", bufs=1, space="PSUM"))

        # --- resident constants ---
        w1_sb = consts.tile([128, E, 2, H1], RDT)
        nc.sync.dma_start(w1_sb[:], w1_d)
        waug_sb = consts.tile([128, E, 2, 80], RDT)
        nc.sync.dma_start(waug_sb[:], waug_d)
        b1_sb = consts.tile([128, 2, E], F32)
        nc.sync.dma_start(b1_sb[:], b1_d)
        b2aug_sb = consts.tile([80, E], F32)
        nc.sync.dma_start(b2aug_sb[:], b2aug_d)
        zc_sb = consts.tile([80, 63], RDT)
        nc.sync.dma_start(zc_sb[:], zc_d)
        i64_sb = consts.tile([64, 64], RDT)
        nc.sync.dma_start(i64_sb[:], i64_d)
        ones64_sb = consts.tile([1, 64], RDT)
        nc.sync.dma_start(ones64_sb[:], ones64_d)
        ident_sb = consts.tile([128, 128], F32)
        nc.sync.dma_start(ident_sb[:], ident_d)
        onescol_sb = consts.tile([128, 1], F32)
        nc.sync.dma_start(onescol_sb[:], onescol_d)

        # mask tile: rows 0:64 ones (static), rows 64:80 onehot(task) per tile
        mmask = consts.tile([80, F], F32)
        nc.vector.memset(mmask[0:64, :], 1.0)
        # reg-loss accumulator
        racc = consts.tile([128, 1], F32)
        nc.vector.memset(racc[:], 0.0)

        for f in range(NF):
            # ---- stage A: load x, transpose to xT [128i, 2k, 512b] ----
            xT = xtp.tile([128, 2, F], RDT, tag="xT")
            for s in range(4):
                x_sb = xp.tile([128, D_IN], F32, tag="x")
                nc.sync.dma_start(x_sb[:], x_t[f, s])
                for k in range(2):
                    pt = pT.tile([128, 128], F32, tag="pt")
                    nc.tensor.transpose(pt[:], x_sb[:, 128 * k : 128 * (k + 1)], ident_sb[:])
                    nc.vector.tensor_copy(xT[:, k, 128 * s : 128 * (s + 1)], pt[:])
            # onehot rows of the mask
            nc.sync.dma_start(mmask[64:80, :], ut_d[:, f * F : (f + 1) * F])

            evs = evp.tile([128, E, F], RDT, tag="evs")
            s32p = pS32.tile([32, F], F32, tag="s32")

            # ---- pass 1 over experts: L1 + VS + mask + select ----
            for e in range(E):
                h_sb = hp.tile([128, 2, F], RDT, tag="h")
                for m in range(2):
                    ph = pH.tile([128, F], F32, tag="ph")
                    for k in range(2):
                        nc.tensor.matmul(
                            ph[:],
                            w1_sb[:, e, k, 128 * m : 128 * (m + 1)],
                            xT[:, k, :],
                            start=(k == 0),
                            stop=(k == 1),
                        )
                    nc.scalar.activation(
                        h_sb[:, m, :], ph[:], ACTF.Relu, bias=b1_sb[:, m, e : e + 1]
                    )
                pvs = pVS.tile([128, F], F32, tag="pvs")
                for k in range(2):
                    nc.tensor.matmul(
                        pvs[0:80, :],
                        waug_sb[:, e, k, :],
                        h_sb[:, k, :],
                        start=(k == 0),
                        stop=(k == 1),
                    )
                # (psum + bias) * mask -> evs[:, e, :]
                nc.vector.scalar_tensor_tensor(
                    evs[0:80, e, :],
                    pvs[0:80, :],
                    b2aug_sb[:, e : e + 1],
                    mmask[:],
                    op0=AL.add,
                    op1=AL.mult,
                )
                # accumulate selected score into S32 row e
                nc.tensor.matmul(
                    s32p[:],
                    zc_sb[64:80, 31 - e : 63 - e],
                    evs[64:80, e, :],
                    start=(e == 0),
                    stop=(e == E - 1),
                    tile_position=(64, 0),
                )

            # ---- stage C: softmax over experts (batch-partition layout) ----
            s32_sb = smp.tile([32, F], F32, tag="s32sb")
            nc.vector.tensor_copy(s32_sb[:], s32p[:])
            wT = smp.tile([32, F], RDT, tag="wT")
            for s in range(4):
                pst = pT.tile([128, 128], F32, tag="pt")
                nc.tensor.transpose(
                    pst[0:128, 0:32],
                    s32_sb[:, 128 * s : 128 * (s + 1)],
                    ident_sb[0:32, 0:32],
                )
                negmx = smp.tile([128, 1], F32, tag="negmx")
                nc.vector.reduce_max(
                    negmx[:], pst[0:128, 0:32], axis=mybir.AxisListType.X, negate=True
                )
                wraw = smp.tile([128, 32], F32, tag="wraw")
                nc.scalar.activation(wraw[:], pst[0:128, 0:32], ACTF.Exp, bias=negmx[:])
                zsum = smp.tile([128, 1], F32, tag="zsum")
                nc.vector.reduce_sum(zsum[:], wraw[:], axis=mybir.AxisListType.X)
                rz = smp.tile([128, 1], F32, tag="rz")
                nc.vector.reciprocal(rz[:], zsum[:])
                w_b = smp.tile([128, 32], F32, tag="wb")
                nc.vector.tensor_scalar_mul(w_b[:], wraw[:], rz[:])
                wsum = smp.tile([128, 1], F32, tag="wsum")
                nc.vector.reduce_sum(wsum[:], w_b[:], axis=mybir.AxisListType.X)
                nc.vector.tensor_add(racc[:], racc[:], wsum[:])
                pwt = pT.tile([128, 128], F32, tag="pt")
                nc.tensor.transpose(pwt[0:32, 0:128], w_b[:], ident_sb[:])
                nc.vector.tensor_copy(wT[:, 128 * s : 128 * (s + 1)], pwt[0:32, 0:128])

            # ---- pass 2: broadcast w, scale values, accumulate yT ----
            pyt = pY.tile([64, F], F32, tag="py")
            for e in range(E):
                wrow = smp.tile([1, F], RDT, tag="wrow")
                nc.sync.dma_start(wrow[:], wT[e : e + 1, :])
                pb = pB.tile([64, F], F32, tag="pb")
                nc.tensor.matmul(pb[:], ones64_sb[:], wrow[:], start=True, stop=True)
                sv = hp.tile([64, F], RDT, tag="sv")
                nc.vector.tensor_mul(sv[:], evs[0:64, e, :], pb[:])
                nc.tensor.matmul(
                    pyt[:], i64_sb[:], sv[:], start=(e == 0), stop=(e == E - 1)
                )
            yT_sb = outp.tile([64, F], F32, tag="yT")
            nc.scalar.copy(yT_sb[:], pyt[:])
            for s in range(4):
                pot = pT.tile([128, 128], F32, tag="pt")
                nc.tensor.transpose(
                    pot[0:128, 0:64],
                    yT_sb[:, 128 * s : 128 * (s + 1)],
                    ident_sb[0:64, 0:64],
                )
                y_sb = outp.tile([128, D_OUT], F32, tag="ysb")
                nc.vector.tensor_copy(y_sb[:], pot[0:128, 0:64])
                nc.sync.dma_start(y_t[f, s], y_sb[:])

        # ---- reg-loss partial: sum racc over partitions ----
        prs = pT.tile([128, 128], F32, tag="pt")
        nc.tensor.matmul(prs[0:1, 0:1], onescol_sb[:], racc[:], start=True, stop=True)
        rs_sb = outp.tile([1, 1], F32, tag="rssb")
        nc.vector.tensor_copy(rs_sb[:], prs[0:1, 0:1])
        nc.sync.dma_start(rs_d, rs_sb[:])

    nc.compile()
    return nc


_NC_CACHE = {}
LAST_EXEC_NS = None


def _get_program():
    key = MATMUL_DTYPE
    if key not in _NC_CACHE:
        _NC_CACHE[key] = _build_program()
    return _NC_CACHE[key]


def kernel(**inputs):
    x = np.asarray(inputs["backbone_output"], dtype=np.float32)
    task = np.asarray(inputs["task"]).astype(np.int64)
    W1 = np.asarray(inputs["W1"], dtype=np.float64)
    b1 = np.asarray(inputs["b1"], dtype=np.float32)
    W2 = np.asarray(inputs["W2"], dtype=np.float64)
    b2 = np.asarray(inputs["b2"], dtype=np.float64)
    TQ = np.asarray(inputs["task_queries"], dtype=np.float64)
    KM = np.asarray(inputs["key_matricies"], dtype=np.float64)
    VM = np.asarray(inputs["value_matricies"], dtype=np.float64)

    # ---- exact algebraic folding (float64) ----
    W2K = np.einsum("eho,eoj->ehj", W2, KM)          # [E,256,64]
    W2V = np.einsum("eho,eoj->ehj", W2, VM)
    G = np.einsum("ehj,tj->eht", W2K, TQ)            # [E,256,16]
    cET = np.einsum("eo,eoj,tj->et", b2, KM, TQ)     # [E,16]
    b2V = np.einsum("eo,eoj->ej", b2, VM)            # [E,64]

    waug = np.concatenate([W2V, G], axis=2)          # [E,256,80]
    waug = np.ascontiguousarray(
        waug.reshape(E, 2, 128, 80).transpose(2, 0, 1, 3)
    ).astype(np.float32)                             # [128,E,2,80]
    w1r = np.ascontiguousarray(
        np.asarray(W1, np.float32).reshape(E, 2, 128, H1).transpose(2, 0, 1, 3)
    )                                                # [128,E,2,H1]
    b1r = np.ascontiguousarray(b1.reshape(E, 2, 128).transpose(2, 1, 0))  # [128,2,E]
    b2aug = np.ascontiguousarray(
        np.concatenate([b2V, cET], axis=1).T
    ).astype(np.float32)                             # [80,E]

    ut = np.zeros((T, B), np.float32)
    ut[task, np.arange(B)] = 1.0

    zc = np.zeros((80, 63), np.float32)
    zc[64:80, 31] = 1.0
    i64 = np.eye(64, dtype=np.float32)
    ones64 = np.ones((1, 64), np.float32)
    ident = np.eye(128, dtype=np.float32)
    onescol = np.ones((128, 1), np.float32)

    shared = {
        "w1": w1r,
        "waug": waug,
        "b1r": b1r,
        "b2aug": b2aug,
        "zc": zc,
        "i64": i64,
        "ones64": ones64,
        "ident": ident,
        "onescol": onescol,
    }
    in_maps = []
    for c in range(NCORES):
        sl = slice(c * BC, (c + 1) * BC)
        in_maps.append({"x": x[sl], "ut": ut[:, sl], **shared})

    nc = _get_program()
    trace = os.environ.get("MOE_TRACE", "0") == "1"
    br = run_bass_kernel_spmd(nc, in_maps, list(range(NCORES)), trace=trace)
    res = br.results
    global LAST_EXEC_NS
    LAST_EXEC_NS = br.exec_time_ns
    if trace and br.instructions_and_trace is not None:
        print("trace:", br.instructions_and_trace[1])

    y = np.concatenate([res[c]["y"] for c in range(NCORES)], axis=0)
    wsum = float(sum(res[c]["rsum"][0, 0] for c in range(NCORES)))
    reg = np.float32(-(MU / E) * (wsum + EPS * B * E))
    return y, reg


# revision 6
# speedup vs baseline: 1.1089x; 1.1089x over previous
"""Trainium2 Bass kernel for nn_MoELayer (dense MoE + attention-over-experts).

Strategy
--------
Data-parallel over batch across 8 NeuronCores (4096 rows/core), params
replicated. Host-side algebraic folding (exact math, done in float64):

    expert_keys   = h @ (W2 Ke) + b2 Ke        (h = relu(x W1 + b1))
    scores[b,e]   = h . G[e, task[b]] + c[e, task[b]],  G[e,t] = (W2 Ke) q_t
    expert_values = h @ (W2 Ve) + b2 Ve

so expert_outputs / keys never materialize. Per 512-column batch tile,
per expert, the device does:
  L1:   hT_e[256,512]  = W1[e]^T-matmul (PE, fp32r)   + relu/bias (ACT)
  VS:   [80,512] psum  = [W2V[e] | G[e]^T]-matmul: rows 0:64 = values^T,
        rows 64:80 = scores vs all 16 task queries
  mask: (psum + bias) * M  (DVE scalar_tensor_tensor), M rows 64:80 = onehot(task)
  sel:  basis matmul accumulates the task-selected score into S32[32,512]
Softmax runs in batch-partition layout via PE transposes; the combine
pre-broadcasts w (K=1 PE outer product), scales values (DVE) and
accumulates sum_e w_e * ev_e on PE into yT, transposed back for output.

float32r (fp32 rounded to 11 mantissa bits, RNE) runs matmuls at 4x the
fp32 rate; set MATMUL_DTYPE = "float32" for the exact-fp32 fallback.
"""

import os
import sys

for _p in ("/opt/trn_rl_repo", "/root/.axon_site/_ro/trn_rl_repo"):
    if os.path.isdir(_p) and _p not in sys.path:
        sys.path.insert(0, _p)

import numpy as np
from contextlib import ExitStack

import concourse.bacc as bacc
import concourse.mybir as mybir
import concourse.tile as tile
from concourse.bass_utils import run_bass_kernel_spmd

F32 = mybir.dt.float32
F32R = mybir.dt.float32r
F16 = mybir.dt.float16
AL = mybir.AluOpType
ACTF = mybir.ActivationFunctionType

B, D_IN, H1, D_OUT = 32768, 256, 256, 64
E, T, MU = 32, 16, 0.01
NCORES = 8
BC = B // NCORES          # 4096 rows per core
F = 512                   # batch columns per tile
NF = BC // F              # 8 tiles per core
EPS = 1e-6

MATMUL_DTYPE = os.environ.get("MOE_MM_DTYPE", "float32r")
RDT = F32R if MATMUL_DTYPE == "float32r" else F32
EVDT = F16 if MATMUL_DTYPE == "float32r" else F32   # expert-value cache dtype
EV_BUFS = 2 if MATMUL_DTYPE == "float32r" else 1


def _build_program():
    nc = bacc.Bacc("TRN2", target_bir_lowering=False, debug=False)

    x_d = nc.dram_tensor("x", [BC, D_IN], F32, kind="ExternalInput").ap()
    ut_d = nc.dram_tensor("ut", [T, BC], F32, kind="ExternalInput").ap()
    w1_d = nc.dram_tensor("w1", [128, E, 2, H1], RDT, kind="ExternalInput").ap()
    waug_d = nc.dram_tensor("waug", [128, E, 2, 80], RDT, kind="ExternalInput").ap()
    b1_d = nc.dram_tensor("b1r", [128, 2, E], F32, kind="ExternalInput").ap()
    b2aug_d = nc.dram_tensor("b2aug", [80, E], F32, kind="ExternalInput").ap()
    zc_d = nc.dram_tensor("zc", [80, 63], RDT, kind="ExternalInput").ap()
    i64_d = nc.dram_tensor("i64", [64, 64], RDT, kind="ExternalInput").ap()
    ones64_d = nc.dram_tensor("ones64", [1, 64], RDT, kind="ExternalInput").ap()
    ident_d = nc.dram_tensor("ident", [128, 128], F32, kind="ExternalInput").ap()
    onescol_d = nc.dram_tensor("onescol", [128, 1], F32, kind="ExternalInput").ap()

    y_d = nc.dram_tensor("y", [BC, D_OUT], F32, kind="ExternalOutput").ap()
    rs_d = nc.dram_tensor("rsum", [1, 1], F32, kind="ExternalOutput").ap()

    x_t = x_d.rearrange("(nf s p) i -> nf s p i", s=4, p=128)     # [NF,4,128,256]
    y_t = y_d.rearrange("(nf s p) j -> nf s p j", s=4, p=128)

    with tile.TileContext(nc) as tc, ExitStack() as ctx:
        consts = ctx.enter_context(tc.tile_pool(name="consts", bufs=1))
        evp = ctx.enter_context(tc.tile_pool(name="evp", bufs=EV_BUFS))
        stp = ctx.enter_context(tc.tile_pool(name="stp", bufs=2))
        yap = ctx.enter_context(tc.tile_pool(name="yap", bufs=2))
        wbp = ctx.enter_context(tc.tile_pool(name="wbp", bufs=2))
        xp = ctx.enter_context(tc.tile_pool(name="xp", bufs=2))
        xtp = ctx.enter_context(tc.tile_pool(name="xtp", bufs=2))
        hp = ctx.enter_context(tc.tile_pool(name="hp", bufs=3))
        smp = ctx.enter_context(tc.tile_pool(name="smp", bufs=2))
        outp = ctx.enter_context(tc.tile_pool(name="outp", bufs=2))

        pH = ctx.enter_context(tc.tile_pool(name="pH", bufs=3, space="PSUM"))
        pVS = ctx.enter_context(tc.tile_pool(name="pVS", bufs=2, space="PSUM"))
        pS32 = ctx.enter_context(tc.tile_pool(name="pS32", bufs=1, space="PSUM"))
        pT = ctx.enter_context(tc.tile_pool(name="pT", bufs=2, space="PSUM"))

        # --- resident constants ---
        w1_sb = consts.tile([128, E, 2, H1], RDT)
        nc.sync.dma_start(w1_sb[:], w1_d)
        waug_sb = consts.tile([128, E, 2, 80], RDT)
        nc.sync.dma_start(waug_sb[:], waug_d)
        b1_sb = consts.tile([128, 2, E], F32)
        nc.sync.dma_start(b1_sb[:], b1_d)
        b2aug_sb = consts.tile([80, E], F32)
        nc.sync.dma_start(b2aug_sb[:], b2aug_d)
        zc_sb = consts.tile([80, 63], RDT)
        nc.sync.dma_start(zc_sb[:], zc_d)
        i64_sb = consts.tile([64, 64], RDT)
        nc.sync.dma_start(i64_sb[:], i64_d)
        ones64_sb = consts.tile([1, 64], RDT)
        nc.sync.dma_start(ones64_sb[:], ones64_d)
        ident_sb = consts.tile([128, 128], F32)
        nc.sync.dma_start(ident_sb[:], ident_d)
        onescol_sb = consts.tile([128, 1], F32)
        nc.sync.dma_start(onescol_sb[:], onescol_d)

        # mask tile: rows 64:80 hold onehot(task) per tile
        mmask = consts.tile([80, F], F32)
        # reg-loss accumulator
        racc = consts.tile([128, 1], F32)
        nc.vector.memset(racc[:], 0.0)

        for f in range(NF):
            # ---- stage A: load x, transpose to xT [128i, 2k, 512b] ----
            xT = xtp.tile([128, 2, F], RDT, tag="xT")
            for s in range(4):
                x_sb = xp.tile([128, D_IN], F32, tag="x")
                nc.sync.dma_start(x_sb[:], x_t[f, s])
                for k in range(2):
                    pt = pT.tile([128, 128], F32, tag="pt")
                    nc.tensor.transpose(pt[:], x_sb[:, 128 * k : 128 * (k + 1)], ident_sb[:])
                    nc.vector.tensor_copy(xT[:, k, 128 * s : 128 * (s + 1)], pt[:])
            # onehot rows of the mask
            nc.sync.dma_start(mmask[64:80, :], ut_d[:, f * F : (f + 1) * F])

            evs = evp.tile([64, E, F], EVDT, tag="evs")
            s32p = pS32.tile([32, F], F32, tag="s32")

            # ---- pass 1 over experts: L1 + VS + mask + select ----
            for e in range(E):
                h_sb = hp.tile([128, 2, F], RDT, tag="h")
                for m in range(2):
                    ph = pH.tile([128, F], F32, tag="ph")
                    for k in range(2):
                        nc.tensor.matmul(
                            ph[:],
                            w1_sb[:, e, k, 128 * m : 128 * (m + 1)],
                            xT[:, k, :],
                            start=(k == 0),
                            stop=(k == 1),
                        )
                    nc.scalar.activation(
                        h_sb[:, m, :], ph[:], ACTF.Relu, bias=b1_sb[:, m, e : e + 1]
                    )
                pvs = pVS.tile([128, F], F32, tag="pvs")
                for k in range(2):
                    nc.tensor.matmul(
                        pvs[0:80, :],
                        waug_sb[:, e, k, :],
                        h_sb[:, k, :],
                        start=(k == 0),
                        stop=(k == 1),
                    )
                # values: psum rows 0:64 + bias -> fp16 cache (ACT)
                nc.scalar.activation(
                    evs[:, e, :],
                    pvs[0:64, :],
                    ACTF.Identity,
                    bias=b2aug_sb[0:64, e : e + 1],
                )
                # scores: (psum rows 64:80 + bias) * onehot -> staging (DVE)
                stmp = stp.tile([80, F], RDT, tag="stmp")
                nc.vector.scalar_tensor_tensor(
                    stmp[64:80, :],
                    pvs[64:80, :],
                    b2aug_sb[64:80, e : e + 1],
                    mmask[64:80, :],
                    op0=AL.add,
                    op1=AL.mult,
                )
                # accumulate selected score into S32 row e
                nc.tensor.matmul(
                    s32p[:],
                    zc_sb[64:80, 31 - e : 63 - e],
                    stmp[64:80, :],
                    start=(e == 0),
                    stop=(e == E - 1),
                    tile_position=(64, 0),
                )

            # ---- stage C: softmax over experts (batch-partition layout) ----
            s32_sb = smp.tile([32, F], F32, tag="s32sb")
            nc.vector.tensor_copy(s32_sb[:], s32p[:])
            wT = smp.tile([32, F], F32, tag="wT")
            for s in range(4):
                pst = pT.tile([128, 128], F32, tag="pt")
                nc.tensor.transpose(
                    pst[0:128, 0:32],
                    s32_sb[:, 128 * s : 128 * (s + 1)],
                    ident_sb[0:32, 0:32],
                )
                negmx = smp.tile([128, 1], F32, tag="negmx")
                nc.vector.reduce_max(
                    negmx[:], pst[0:128, 0:32], axis=mybir.AxisListType.X, negate=True
                )
                wraw = smp.tile([128, 32], F32, tag="wraw")
                nc.scalar.activation(wraw[:], pst[0:128, 0:32], ACTF.Exp, bias=negmx[:])
                zsum = smp.tile([128, 1], F32, tag="zsum")
                nc.vector.reduce_sum(zsum[:], wraw[:], axis=mybir.AxisListType.X)
                rz = smp.tile([128, 1], F32, tag="rz")
                nc.vector.reciprocal(rz[:], zsum[:])
                w_b = smp.tile([128, 32], F32, tag="wb")
                nc.vector.tensor_scalar_mul(w_b[:], wraw[:], rz[:])
                wsum = smp.tile([128, 1], F32, tag="wsum")
                nc.vector.reduce_sum(wsum[:], w_b[:], axis=mybir.AxisListType.X)
                nc.vector.tensor_add(racc[:], racc[:], wsum[:])
                pwt = pT.tile([128, 128], F32, tag="pt")
                nc.tensor.transpose(pwt[0:32, 0:128], w_b[:], ident_sb[:])
                nc.vector.tensor_copy(wT[:, 128 * s : 128 * (s + 1)], pwt[0:32, 0:128])

            # ---- pass 2: broadcast w (gpsimd), scale+accumulate (DVE) ----
            yT_sb = yap.tile([64, F], F32, tag="yT")
            for e in range(E):
                wrow = smp.tile([1, F], F32, tag="wrow")
                nc.sync.dma_start(wrow[:], wT[e : e + 1, :])
                wbc = wbp.tile([64, F], F32, tag="wbc")
                nc.gpsimd.partition_broadcast(wbc[:], wrow[:])
                sv = hp.tile([64, F], F32, tag="sv")
                nc.vector.tensor_mul(sv[:], evs[:, e, :], wbc[:])
                if e == 0:
                    nc.vector.tensor_copy(yT_sb[:], sv[:])
                else:
                    nc.vector.tensor_add(yT_sb[:], yT_sb[:], sv[:])
            for s in range(4):
                pot = pT.tile([128, 128], F32, tag="pt")
                nc.tensor.transpose(
                    pot[0:128, 0:64],
                    yT_sb[:, 128 * s : 128 * (s + 1)],
                    ident_sb[0:64, 0:64],
                )
                y_sb = outp.tile([128, D_OUT], F32, tag="ysb")
                nc.vector.tensor_copy(y_sb[:], pot[0:128, 0:64])
                nc.sync.dma_start(y_t[f, s], y_sb[:])

        # ---- reg-loss partial: sum racc over partitions ----
        prs = pT.tile([128, 128], F32, tag="pt")
        nc.tensor.matmul(prs[0:1, 0:1], onescol_sb[:], racc[:], start=True, stop=True)
        rs_sb = outp.tile([1, 1], F32, tag="rssb")
        nc.vector.tensor_copy(rs_sb[:], prs[0:1, 0:1])
        nc.sync.dma_start(rs_d, rs_sb[:])

    nc.compile()
    return nc


_NC_CACHE = {}
LAST_EXEC_NS = None


def _get_program():
    key = MATMUL_DTYPE
    if key not in _NC_CACHE:
        _NC_CACHE[key] = _build_program()
    return _NC_CACHE[key]


def kernel(**inputs):
    x = np.asarray(inputs["backbone_output"], dtype=np.float32)
    task = np.asarray(inputs["task"]).astype(np.int64)
    W1 = np.asarray(inputs["W1"], dtype=np.float64)
    b1 = np.asarray(inputs["b1"], dtype=np.float32)
    W2 = np.asarray(inputs["W2"], dtype=np.float64)
    b2 = np.asarray(inputs["b2"], dtype=np.float64)
    TQ = np.asarray(inputs["task_queries"], dtype=np.float64)
    KM = np.asarray(inputs["key_matricies"], dtype=np.float64)
    VM = np.asarray(inputs["value_matricies"], dtype=np.float64)

    # ---- exact algebraic folding (float64) ----
    W2K = np.einsum("eho,eoj->ehj", W2, KM)          # [E,256,64]
    W2V = np.einsum("eho,eoj->ehj", W2, VM)
    G = np.einsum("ehj,tj->eht", W2K, TQ)            # [E,256,16]
    cET = np.einsum("eo,eoj,tj->et", b2, KM, TQ)     # [E,16]
    b2V = np.einsum("eo,eoj->ej", b2, VM)            # [E,64]

    waug = np.concatenate([W2V, G], axis=2)          # [E,256,80]
    waug = np.ascontiguousarray(
        waug.reshape(E, 2, 128, 80).transpose(2, 0, 1, 3)
    ).astype(np.float32)                             # [128,E,2,80]
    w1r = np.ascontiguousarray(
        np.asarray(W1, np.float32).reshape(E, 2, 128, H1).transpose(2, 0, 1, 3)
    )                                                # [128,E,2,H1]
    b1r = np.ascontiguousarray(b1.reshape(E, 2, 128).transpose(2, 1, 0))  # [128,2,E]
    b2aug = np.ascontiguousarray(
        np.concatenate([b2V, cET], axis=1).T
    ).astype(np.float32)                             # [80,E]

    ut = np.zeros((T, B), np.float32)
    ut[task, np.arange(B)] = 1.0

    zc = np.zeros((80, 63), np.float32)
    zc[64:80, 31] = 1.0
    i64 = np.eye(64, dtype=np.float32)
    ones64 = np.ones((1, 64), np.float32)
    ident = np.eye(128, dtype=np.float32)
    onescol = np.ones((128, 1), np.float32)

    shared = {
        "w1": w1r,
        "waug": waug,
        "b1r": b1r,
        "b2aug": b2aug,
        "zc": zc,
        "i64": i64,
        "ones64": ones64,
        "ident": ident,
        "onescol": onescol,
    }
    in_maps = []
    for c in range(NCORES):
        sl = slice(c * BC, (c + 1) * BC)
        in_maps.append({"x": x[sl], "ut": ut[:, sl], **shared})

    nc = _get_program()
    trace = os.environ.get("MOE_TRACE", "0") == "1"
    br = run_bass_kernel_spmd(nc, in_maps, list(range(NCORES)), trace=trace)
    res = br.results
    global LAST_EXEC_NS
    LAST_EXEC_NS = br.exec_time_ns
    if trace and br.instructions_and_trace is not None:
        print("trace:", br.instructions_and_trace[1])

    y = np.concatenate([res[c]["y"] for c in range(NCORES)], axis=0)
    wsum = float(sum(res[c]["rsum"][0, 0] for c in range(NCORES)))
    reg = np.float32(-(MU / E) * (wsum + EPS * B * E))
    return y, reg


# revision 8
# speedup vs baseline: 1.4037x; 1.2658x over previous
"""Trainium2 Bass kernel for nn_MoELayer (dense MoE + attention-over-experts).

Strategy
--------
Data-parallel over batch across 8 NeuronCores (4096 rows/core), params
replicated. Host-side algebraic folding (exact math, done in float64):

    expert_keys   = h @ (W2 Ke) + b2 Ke        (h = relu(x W1 + b1))
    scores[b,e]   = h . G[e, task[b]] + c[e, task[b]],  G[e,t] = (W2 Ke) q_t
    expert_values = h @ (W2 Ve) + b2 Ve

so expert_outputs / keys never materialize. Per 512-column batch tile,
per expert, the device does:
  L1:   hT_e[256,512]  = W1[e]^T-matmul (PE, fp32r)   + relu/bias (ACT)
  VS:   [80,512] psum  = [W2V[e] | G[e]^T]-matmul: rows 0:64 = values^T,
        rows 64:80 = scores vs all 16 task queries
  mask: (psum + bias) * M  (DVE scalar_tensor_tensor), M rows 64:80 = onehot(task)
  sel:  basis matmul accumulates the task-selected score into S32[32,512]
Softmax runs in batch-partition layout via PE transposes; the combine
pre-broadcasts w (K=1 PE outer product), scales values (DVE) and
accumulates sum_e w_e * ev_e on PE into yT, transposed back for output.

float32r (fp32 rounded to 11 mantissa bits, RNE) runs matmuls at 4x the
fp32 rate; set MATMUL_DTYPE = "float32" for the exact-fp32 fallback.
"""

import os
import sys

for _p in ("/opt/trn_rl_repo", "/root/.axon_site/_ro/trn_rl_repo"):
    if os.path.isdir(_p) and _p not in sys.path:
        sys.path.insert(0, _p)

import numpy as np
from contextlib import ExitStack

import concourse.bacc as bacc
import concourse.mybir as mybir
import concourse.tile as tile
from concourse.bass_utils import run_bass_kernel_spmd

F32 = mybir.dt.float32
F32R = mybir.dt.float32r
F16 = mybir.dt.float16
AL = mybir.AluOpType
ACTF = mybir.ActivationFunctionType

B, D_IN, H1, D_OUT = 32768, 256, 256, 64
E, T, MU = 32, 16, 0.01
NCORES = 8
BC = B // NCORES          # 4096 rows per core
F = 512                   # batch columns per tile
NF = BC // F              # 8 tiles per core
EPS = 1e-6

MATMUL_DTYPE = os.environ.get("MOE_MM_DTYPE", "float32r")
RDT = F32R if MATMUL_DTYPE == "float32r" else F32
EVDT = F16 if MATMUL_DTYPE == "float32r" else F32   # expert-value cache dtype
EV_BUFS = 2 if MATMUL_DTYPE == "float32r" else 1


def _build_program():
    nc = bacc.Bacc("TRN2", target_bir_lowering=False, debug=False)

    x_d = nc.dram_tensor("x", [BC, D_IN], F32, kind="ExternalInput").ap()
    ut_d = nc.dram_tensor("ut", [T, BC], F32, kind="ExternalInput").ap()
    w1_d = nc.dram_tensor("w1", [128, E, 2, H1], RDT, kind="ExternalInput").ap()
    waug_d = nc.dram_tensor("waug", [128, E, 2, 80], RDT, kind="ExternalInput").ap()
    b1_d = nc.dram_tensor("b1r", [128, 2, E], F32, kind="ExternalInput").ap()
    b2aug_d = nc.dram_tensor("b2aug", [80, E], F32, kind="ExternalInput").ap()
    zc_d = nc.dram_tensor("zc", [80, 63], RDT, kind="ExternalInput").ap()
    i64_d = nc.dram_tensor("i64", [64, 64], RDT, kind="ExternalInput").ap()
    ones64_d = nc.dram_tensor("ones64", [1, 64], F16, kind="ExternalInput").ap()
    ident_d = nc.dram_tensor("ident", [128, 128], F32, kind="ExternalInput").ap()
    onescol_d = nc.dram_tensor("onescol", [128, 1], F32, kind="ExternalInput").ap()

    y_d = nc.dram_tensor("y", [BC, D_OUT], F32, kind="ExternalOutput").ap()
    rs_d = nc.dram_tensor("rsum", [1, 1], F32, kind="ExternalOutput").ap()

    x_t = x_d.rearrange("(nf s p) i -> nf s p i", s=4, p=128)     # [NF,4,128,256]
    y_t = y_d.rearrange("(nf s p) j -> nf s p j", s=4, p=128)

    with tile.TileContext(nc) as tc, ExitStack() as ctx:
        consts = ctx.enter_context(tc.tile_pool(name="consts", bufs=1))
        evp = ctx.enter_context(tc.tile_pool(name="evp", bufs=EV_BUFS))
        stp = ctx.enter_context(tc.tile_pool(name="stp", bufs=2))
        yap = ctx.enter_context(tc.tile_pool(name="yap", bufs=2))
        wbp = ctx.enter_context(tc.tile_pool(name="wbp", bufs=2))
        xp = ctx.enter_context(tc.tile_pool(name="xp", bufs=2))
        xtp = ctx.enter_context(tc.tile_pool(name="xtp", bufs=2))
        hp = ctx.enter_context(tc.tile_pool(name="hp", bufs=3))
        smp = ctx.enter_context(tc.tile_pool(name="smp", bufs=2))
        outp = ctx.enter_context(tc.tile_pool(name="outp", bufs=2))

        pH = ctx.enter_context(tc.tile_pool(name="pH", bufs=2, space="PSUM"))
        pVS = ctx.enter_context(tc.tile_pool(name="pVS", bufs=2, space="PSUM"))
        pS32 = ctx.enter_context(tc.tile_pool(name="pS32", bufs=1, space="PSUM"))
        pT = ctx.enter_context(tc.tile_pool(name="pT", bufs=2, space="PSUM"))
        pB = ctx.enter_context(tc.tile_pool(name="pB", bufs=1, space="PSUM"))

        # --- resident constants ---
        w1_sb = consts.tile([128, E, 2, H1], RDT)
        nc.sync.dma_start(w1_sb[:], w1_d)
        waug_sb = consts.tile([128, E, 2, 80], RDT)
        nc.sync.dma_start(waug_sb[:], waug_d)
        b1_sb = consts.tile([128, 2, E], F32)
        nc.sync.dma_start(b1_sb[:], b1_d)
        b2aug_sb = consts.tile([80, E], F32)
        nc.sync.dma_start(b2aug_sb[:], b2aug_d)
        zc_sb = consts.tile([80, 63], RDT)
        nc.sync.dma_start(zc_sb[:], zc_d)
        i64_sb = consts.tile([64, 64], RDT)
        nc.sync.dma_start(i64_sb[:], i64_d)
        ones64_sb = consts.tile([1, 64], F16)
        nc.sync.dma_start(ones64_sb[:], ones64_d)
        ident_sb = consts.tile([128, 128], F32)
        nc.sync.dma_start(ident_sb[:], ident_d)
        onescol_sb = consts.tile([128, 1], F32)
        nc.sync.dma_start(onescol_sb[:], onescol_d)

        # mask tile: rows 64:80 hold onehot(task) per tile
        mmask = consts.tile([80, F], F32)
        # reg-loss accumulator
        racc = consts.tile([128, 1], F32)
        nc.vector.memset(racc[:], 0.0)

        ytiles = []

        def emit_tail():
            ft, yt = ytiles.pop(0)
            for s in range(4):
                pot = pT.tile([128, 128], F32, tag="pt")
                nc.tensor.transpose(
                    pot[0:128, 0:64],
                    yt[:, 128 * s : 128 * (s + 1)],
                    ident_sb[0:64, 0:64],
                )
                y_sb = outp.tile([128, D_OUT], F32, tag="ysb")
                nc.vector.tensor_copy(y_sb[:], pot[0:128, 0:64])
                nc.sync.dma_start(y_t[ft, s], y_sb[:])

        for f in range(NF):
            # ---- stage A: load x, transpose to xT [128i, 2k, 512b] ----
            xT = xtp.tile([128, 2, F], RDT, tag="xT")
            for s in range(4):
                x_sb = xp.tile([128, D_IN], F32, tag="x")
                nc.sync.dma_start(x_sb[:], x_t[f, s])
                for k in range(2):
                    pt = pT.tile([128, 128], F32, tag="pt")
                    nc.tensor.transpose(pt[:], x_sb[:, 128 * k : 128 * (k + 1)], ident_sb[:])
                    nc.vector.tensor_copy(xT[:, k, 128 * s : 128 * (s + 1)], pt[:])
            # onehot rows of the mask
            nc.sync.dma_start(mmask[64:80, :], ut_d[:, f * F : (f + 1) * F])

            evs = evp.tile([64, E, F], EVDT, tag="evs")
            s32p = pS32.tile([32, F], F32, tag="s32")

            # ---- pass 1 over experts: L1 + VS + mask + select ----
            for e in range(E):
                h_sb = hp.tile([128, 2, F], RDT, tag="h")
                for m in range(2):
                    ph = pH.tile([128, F], F32, tag="ph")
                    for k in range(2):
                        nc.tensor.matmul(
                            ph[:],
                            w1_sb[:, e, k, 128 * m : 128 * (m + 1)],
                            xT[:, k, :],
                            start=(k == 0),
                            stop=(k == 1),
                        )
                    nc.scalar.activation(
                        h_sb[:, m, :], ph[:], ACTF.Relu, bias=b1_sb[:, m, e : e + 1]
                    )
                pvs = pVS.tile([128, F], F32, tag="pvs")
                for k in range(2):
                    nc.tensor.matmul(
                        pvs[0:80, :],
                        waug_sb[:, e, k, :],
                        h_sb[:, k, :],
                        start=(k == 0),
                        stop=(k == 1),
                    )
                # values: psum rows 0:64 + bias -> fp16 cache (ACT)
                nc.scalar.activation(
                    evs[:, e, :],
                    pvs[0:64, :],
                    ACTF.Identity,
                    bias=b2aug_sb[0:64, e : e + 1],
                )
                # scores: (psum rows 64:80 + bias) * onehot -> staging (DVE)
                stmp = stp.tile([80, F], RDT, tag="stmp")
                nc.vector.scalar_tensor_tensor(
                    stmp[64:80, :],
                    pvs[64:80, :],
                    b2aug_sb[64:80, e : e + 1],
                    mmask[64:80, :],
                    op0=AL.add,
                    op1=AL.mult,
                )
                # accumulate selected score into S32 row e
                nc.tensor.matmul(
                    s32p[:],
                    zc_sb[64:80, 31 - e : 63 - e],
                    stmp[64:80, :],
                    start=(e == 0),
                    stop=(e == E - 1),
                    tile_position=(64, 0),
                )

            if ytiles:
                emit_tail()

            # ---- stage C: softmax over experts (batch-partition layout) ----
            s32_sb = smp.tile([32, F], F32, tag="s32sb")
            nc.vector.tensor_copy(s32_sb[:], s32p[:])
            wT = smp.tile([32, F], F16, tag="wT")
            for s in range(4):
                pst = pT.tile([128, 128], F32, tag="pt")
                nc.tensor.transpose(
                    pst[0:128, 0:32],
                    s32_sb[:, 128 * s : 128 * (s + 1)],
                    ident_sb[0:32, 0:32],
                )
                negmx = smp.tile([128, 1], F32, tag="negmx")
                nc.vector.reduce_max(
                    negmx[:], pst[0:128, 0:32], axis=mybir.AxisListType.X, negate=True
                )
                wraw = smp.tile([128, 32], F32, tag="wraw")
                nc.scalar.activation(wraw[:], pst[0:128, 0:32], ACTF.Exp, bias=negmx[:])
                zsum = smp.tile([128, 1], F32, tag="zsum")
                nc.vector.reduce_sum(zsum[:], wraw[:], axis=mybir.AxisListType.X)
                rz = smp.tile([128, 1], F32, tag="rz")
                nc.vector.reciprocal(rz[:], zsum[:])
                w_b = smp.tile([128, 32], F32, tag="wb")
                nc.vector.tensor_scalar_mul(w_b[:], wraw[:], rz[:])
                wsum = smp.tile([128, 1], F32, tag="wsum")
                nc.vector.reduce_sum(wsum[:], w_b[:], axis=mybir.AxisListType.X)
                nc.vector.tensor_add(racc[:], racc[:], wsum[:])
                pwt = pT.tile([128, 128], F32, tag="pt")
                nc.tensor.transpose(pwt[0:32, 0:128], w_b[:], ident_sb[:])
                nc.vector.tensor_copy(wT[:, 128 * s : 128 * (s + 1)], pwt[0:32, 0:128])

            # ---- pass 2: broadcast w (PE fp16), scale + accumulate (DVE) ----
            yT_sb = yap.tile([64, F], F32, tag="yT")
            for e in range(E):
                wrow = smp.tile([1, F], F16, tag="wrow")
                nc.sync.dma_start(wrow[:], wT[e : e + 1, :])
                pb = pB.tile([64, F], F32, tag="pb")
                nc.tensor.matmul(pb[:], ones64_sb[:], wrow[:], start=True, stop=True)
                sv = hp.tile([64, F], F32, tag="sv")
                nc.vector.tensor_mul(sv[:], evs[:, e, :], pb[:])
                if e == 0:
                    nc.vector.tensor_copy(yT_sb[:], sv[:])
                else:
                    nc.vector.tensor_add(yT_sb[:], yT_sb[:], sv[:])
            ytiles.append((f, yT_sb))

        while ytiles:
            emit_tail()

        # ---- reg-loss partial: sum racc over partitions ----
        prs = pT.tile([128, 128], F32, tag="pt")
        nc.tensor.matmul(prs[0:1, 0:1], onescol_sb[:], racc[:], start=True, stop=True)
        rs_sb = outp.tile([1, 1], F32, tag="rssb")
        nc.vector.tensor_copy(rs_sb[:], prs[0:1, 0:1])
        nc.sync.dma_start(rs_d, rs_sb[:])

    nc.compile()
    return nc


_NC_CACHE = {}
LAST_EXEC_NS = None


def _get_program():
    key = MATMUL_DTYPE
    if key not in _NC_CACHE:
        _NC_CACHE[key] = _build_program()
    return _NC_CACHE[key]


def kernel(**inputs):
    x = np.asarray(inputs["backbone_output"], dtype=np.float32)
    task = np.asarray(inputs["task"]).astype(np.int64)
    W1 = np.asarray(inputs["W1"], dtype=np.float64)
    b1 = np.asarray(inputs["b1"], dtype=np.float32)
    W2 = np.asarray(inputs["W2"], dtype=np.float64)
    b2 = np.asarray(inputs["b2"], dtype=np.float64)
    TQ = np.asarray(inputs["task_queries"], dtype=np.float64)
    KM = np.asarray(inputs["key_matricies"], dtype=np.float64)
    VM = np.asarray(inputs["value_matricies"], dtype=np.float64)

    # ---- exact algebraic folding (float64) ----
    W2K = np.einsum("eho,eoj->ehj", W2, KM)          # [E,256,64]
    W2V = np.einsum("eho,eoj->ehj", W2, VM)
    G = np.einsum("ehj,tj->eht", W2K, TQ)            # [E,256,16]
    cET = np.einsum("eo,eoj,tj->et", b2, KM, TQ)     # [E,16]
    b2V = np.einsum("eo,eoj->ej", b2, VM)            # [E,64]

    waug = np.concatenate([W2V, G], axis=2)          # [E,256,80]
    waug = np.ascontiguousarray(
        waug.reshape(E, 2, 128, 80).transpose(2, 0, 1, 3)
    ).astype(np.float32)                             # [128,E,2,80]
    w1r = np.ascontiguousarray(
        np.asarray(W1, np.float32).reshape(E, 2, 128, H1).transpose(2, 0, 1, 3)
    )                                                # [128,E,2,H1]
    b1r = np.ascontiguousarray(b1.reshape(E, 2, 128).transpose(2, 1, 0))  # [128,2,E]
    b2aug = np.ascontiguousarray(
        np.concatenate([b2V, cET], axis=1).T
    ).astype(np.float32)                             # [80,E]

    ut = np.zeros((T, B), np.float32)
    ut[task, np.arange(B)] = 1.0

    zc = np.zeros((80, 63), np.float32)
    zc[64:80, 31] = 1.0
    i64 = np.eye(64, dtype=np.float32)
    ones64 = np.ones((1, 64), np.float16)
    ident = np.eye(128, dtype=np.float32)
    onescol = np.ones((128, 1), np.float32)

    shared = {
        "w1": w1r,
        "waug": waug,
        "b1r": b1r,
        "b2aug": b2aug,
        "zc": zc,
        "i64": i64,
        "ones64": ones64,
        "ident": ident,
        "onescol": onescol,
    }
    in_maps = []
    for c in range(NCORES):
        sl = slice(c * BC, (c + 1) * BC)
        in_maps.append({"x": x[sl], "ut": ut[:, sl], **shared})

    nc = _get_program()
    trace = os.environ.get("MOE_TRACE", "0") == "1"
    br = run_bass_kernel_spmd(nc, in_maps, list(range(NCORES)), trace=trace)
    res = br.results
    global LAST_EXEC_NS
    LAST_EXEC_NS = br.exec_time_ns
    if trace and br.instructions_and_trace is not None:
        print("trace:", br.instructions_and_trace[1])

    y = np.concatenate([res[c]["y"] for c in range(NCORES)], axis=0)
    wsum = float(sum(res[c]["rsum"][0, 0] for c in range(NCORES)))
    reg = np.float32(-(MU / E) * (wsum + EPS * B * E))
    return y, reg


# revision 14
# speedup vs baseline: 1.8584x; 1.3240x over previous
"""Trainium2 Bass kernel for nn_MoELayer (dense MoE + attention-over-experts).

Strategy
--------
Data-parallel over batch across 8 NeuronCores (4096 rows/core), params
replicated. Host-side algebraic folding (exact math, done in float64):

    expert_keys   = h @ (W2 Ke) + b2 Ke        (h = relu(x W1 + b1))
    scores[b,e]   = h . G[e, task[b]] + c[e, task[b]],  G[e,t] = (W2 Ke) q_t
    expert_values = h @ (W2 Ve) + b2 Ve

so expert_outputs / keys never materialize. Per 512-column batch tile,
per expert, the device does:
  L1:   hT_e[256,512]  = W1[e]^T-matmul (PE, fp32r) + relu/bias (ACT)
  VS:   [128,512] psum = [W2V[e] | G[e]^T @ col 64+16(e%4)]-matmul:
        rows 0:64 = values^T (fp16 cache), rows 64+16q:80+16q = scores
        vs the 16 task queries (placement encoded in the weight matrix)
  mask: (psum + bias) * onehot(task)  (DVE) into a 4-expert staging tile
  sel:  one basis matmul per 4 experts accumulates selected scores into
        S32[32,512]
Softmax runs in batch-partition layout via PE transposes; the combine
broadcasts w via fp16 K=1 PE outer products, then scales/accumulates on
DVE.  The expert loop is software-pipelined (L1 one expert ahead) and
each tile's output transposes are deferred behind the next tile's dense
phase so the PE never drains.

float32r (fp32 rounded to 11 mantissa bits RNE, 2 PE cycles/row) runs
4x faster than fp32; MOE_MM_DTYPE=float32 selects the exact-fp32 build.
"""

import os
import sys

for _p in ("/opt/trn_rl_repo", "/root/.axon_site/_ro/trn_rl_repo"):
    if os.path.isdir(_p) and _p not in sys.path:
        sys.path.insert(0, _p)

import numpy as np
from contextlib import ExitStack

import concourse.bacc as bacc
import concourse.mybir as mybir
import concourse.tile as tile
from concourse.bass_utils import run_bass_kernel_spmd

F32 = mybir.dt.float32
F32R = mybir.dt.float32r
F16 = mybir.dt.float16
AL = mybir.AluOpType
ACTF = mybir.ActivationFunctionType

B, D_IN, H1, D_OUT = 32768, 256, 256, 64
E, T, MU = 32, 16, 0.01
NCORES = 8
BC = B // NCORES          # 4096 rows per core
F = 512                   # batch columns per tile
NF = BC // F              # 8 tiles per core
EPS = 1e-6

MATMUL_DTYPE = os.environ.get("MOE_MM_DTYPE", "float32r")
RDT = F32R if MATMUL_DTYPE == "float32r" else F32
EVDT = F16 if MATMUL_DTYPE == "float32r" else F32   # expert-value cache dtype
EV_BUFS = 2 if MATMUL_DTYPE == "float32r" else 1

LAST_EXEC_NS = None


def _build_program():
    nc = bacc.Bacc("TRN2", target_bir_lowering=False, debug=False)

    x_d = nc.dram_tensor("xt", [2, 128, BC], RDT, kind="ExternalInput").ap()
    ut_d = nc.dram_tensor("ut4", [64, BC], F32, kind="ExternalInput").ap()
    w1_d = nc.dram_tensor("w1", [128, E, 2, H1], RDT, kind="ExternalInput").ap()
    waug_d = nc.dram_tensor("waug", [128, E, 2, 128], RDT, kind="ExternalInput").ap()
    b1_d = nc.dram_tensor("b1r", [128, 2, E], F32, kind="ExternalInput").ap()
    b2aug_d = nc.dram_tensor("b2aug", [128, E], F32, kind="ExternalInput").ap()
    zc4_d = nc.dram_tensor("zc4", [128, 62], RDT, kind="ExternalInput").ap()
    ones64_d = nc.dram_tensor("ones64", [1, 64], F16, kind="ExternalInput").ap()
    ident_d = nc.dram_tensor("ident", [128, 128], F32, kind="ExternalInput").ap()
    onescol_d = nc.dram_tensor("onescol", [128, 1], F32, kind="ExternalInput").ap()

    y_d = nc.dram_tensor("y", [BC, D_OUT], F32, kind="ExternalOutput").ap()
    rs_d = nc.dram_tensor("rsum", [1, 1], F32, kind="ExternalOutput").ap()

    y_t = y_d.rearrange("(nf s p) j -> nf s p j", s=4, p=128)

    with tile.TileContext(nc) as tc, ExitStack() as ctx:
        consts = ctx.enter_context(tc.tile_pool(name="consts", bufs=1))
        evp = ctx.enter_context(tc.tile_pool(name="evp", bufs=EV_BUFS))
        stp = ctx.enter_context(tc.tile_pool(name="stp", bufs=2))
        yap = ctx.enter_context(tc.tile_pool(name="yap", bufs=2))
        xp = ctx.enter_context(tc.tile_pool(name="xp", bufs=2))
        xtp = ctx.enter_context(tc.tile_pool(name="xtp", bufs=2))
        hp = ctx.enter_context(tc.tile_pool(name="hp", bufs=2))
        smp = ctx.enter_context(tc.tile_pool(name="smp", bufs=2))
        outp = ctx.enter_context(tc.tile_pool(name="outp", bufs=2))

        pH = ctx.enter_context(tc.tile_pool(name="pH", bufs=2, space="PSUM"))
        pVS = ctx.enter_context(tc.tile_pool(name="pVS", bufs=2, space="PSUM"))
        pS32 = ctx.enter_context(tc.tile_pool(name="pS32", bufs=1, space="PSUM"))
        pT = ctx.enter_context(tc.tile_pool(name="pT", bufs=2, space="PSUM"))
        pB = ctx.enter_context(tc.tile_pool(name="pB", bufs=1, space="PSUM"))

        # --- small constants first (sync queue), bulk weights on SWDGE ---
        ident_sb = consts.tile([128, 128], F32)
        nc.sync.dma_start(ident_sb[:], ident_d)
        b1_sb = consts.tile([128, 2, E], F32)
        nc.sync.dma_start(b1_sb[:], b1_d)
        b2aug_sb = consts.tile([128, E], F32)
        nc.sync.dma_start(b2aug_sb[:], b2aug_d)
        zc4_sb = consts.tile([128, 62], RDT)
        nc.sync.dma_start(zc4_sb[:], zc4_d)
        ones64_sb = consts.tile([1, 64], F16)
        nc.sync.dma_start(ones64_sb[:], ones64_d)
        onescol_sb = consts.tile([128, 1], F32)
        nc.sync.dma_start(onescol_sb[:], onescol_d)

        # first tile's inputs land before the bulk weights
        xT0 = xtp.tile([128, 2, F], RDT, tag="xT")
        for k in range(2):
            nc.sync.dma_start(xT0[:, k, :], x_d[k, :, 0:F])
        # mask tile: rows 64:128 hold onehot(task) replicated 4x, per tile
        mmask = consts.tile([128, F], F32)
        nc.sync.dma_start(mmask[64:128, :], ut_d[:, 0:F])

        # weights in consumption order: w1[e] just ahead of waug[e]
        w1_sb = consts.tile([128, E, 2, H1], RDT)
        waug_sb = consts.tile([128, E, 2, 128], RDT)
        nc.sync.dma_start(w1_sb[:, 0:2], w1_d[:, 0:2])
        for eq in range(1, 16):
            nc.gpsimd.dma_start(
                w1_sb[:, 2 * eq : 2 * (eq + 1)], w1_d[:, 2 * eq : 2 * (eq + 1)]
            )
            nc.gpsimd.dma_start(
                waug_sb[:, 2 * (eq - 1) : 2 * eq], waug_d[:, 2 * (eq - 1) : 2 * eq]
            )
        nc.gpsimd.dma_start(waug_sb[:, 30:32], waug_d[:, 30:32])
        # reg-loss accumulator
        racc = consts.tile([128, 1], F32)
        nc.vector.memset(racc[:], 0.0)

        ytiles = []

        def emit_tail():
            ft, yt = ytiles.pop(0)
            for s in range(4):
                pot = pT.tile([128, 128], F32, tag="pt")
                nc.tensor.transpose(
                    pot[0:128, 0:64],
                    yt[:, 128 * s : 128 * (s + 1)],
                    ident_sb[0:64, 0:64],
                )
                y_sb = outp.tile([128, D_OUT], F32, tag="ysb")
                nc.vector.tensor_copy(y_sb[:], pot[0:128, 0:64])
                nc.sync.dma_start(y_t[ft, s], y_sb[:])

        for f in range(NF):
            # ---- stage A: load pre-transposed xT [128i, 2k, 512b] ----
            if f == 0:
                xT = xT0
            else:
                xT = xtp.tile([128, 2, F], RDT, tag="xT")
                for k in range(2):
                    nc.sync.dma_start(
                        xT[:, k, :], x_d[k, :, f * F : (f + 1) * F]
                    )
                # onehot rows of the mask (4 replicas)
                nc.sync.dma_start(mmask[64:128, :], ut_d[:, f * F : (f + 1) * F])

            evs = evp.tile([64, E, F], EVDT, tag="evs")
            s32p = pS32.tile([32, F], F32, tag="s32")

            # ---- pass 1 over experts (skewed software pipeline) ----
            hq = {}
            stmp4 = None
            for e in range(E + 1):
                if e < E:
                    h_sb = hp.tile([128, 2, F], RDT, tag="h")
                    for m in range(2):
                        ph = pH.tile([128, F], F32, tag="ph")
                        for k in range(2):
                            nc.tensor.matmul(
                                ph[:],
                                w1_sb[:, e, k, 128 * m : 128 * (m + 1)],
                                xT[:, k, :],
                                start=(k == 0),
                                stop=(k == 1),
                            )
                        nc.scalar.activation(
                            h_sb[:, m, :], ph[:], ACTF.Relu,
                            bias=b1_sb[:, m, e : e + 1],
                        )
                    hq[e] = h_sb
                if e >= 1:
                    ev = e - 1
                    q = ev % 2
                    h_sb = hq.pop(ev)
                    pvs = pVS.tile([128, F], F32, tag="pvs")
                    for k in range(2):
                        nc.tensor.matmul(
                            pvs[:],
                            waug_sb[:, ev, k, :],
                            h_sb[:, k, :],
                            start=(k == 0),
                            stop=(k == 1),
                        )
                    # values: psum rows 0:64 + bias -> fp16 cache (ACT)
                    nc.scalar.activation(
                        evs[:, ev, :],
                        pvs[0:64, :],
                        ACTF.Identity,
                        bias=b2aug_sb[0:64, ev : ev + 1],
                    )
                    # scores: (psum + bias) * onehot -> 2-expert staging (DVE)
                    if q == 0:
                        stmp4 = stp.tile([128, F], RDT, tag="stmp")
                    r0, r1 = 64 + 32 * q, 96 + 32 * q
                    nc.vector.scalar_tensor_tensor(
                        stmp4[r0:r1, :],
                        pvs[r0:r1, :],
                        b2aug_sb[r0:r1, ev : ev + 1],
                        mmask[r0:r1, :],
                        op0=AL.add,
                        op1=AL.mult,
                    )
                    if q == 1:
                        Q = ev // 2
                        nc.tensor.matmul(
                            s32p[:],
                            zc4_sb[64:128, 30 - 2 * Q : 62 - 2 * Q],
                            stmp4[64:128, :],
                            start=(Q == 0),
                            stop=(Q == 15),
                            tile_position=(64, 0),
                        )

            if ytiles:
                emit_tail()

            # ---- stage C: softmax over experts (batch-partition layout) ----
            s32_sb = smp.tile([32, F], F32, tag="s32sb")
            nc.vector.tensor_copy(s32_sb[:], s32p[:])
            wT = smp.tile([32, F], F16, tag="wT")
            for s in range(4):
                pst = pT.tile([128, 128], F32, tag="pt")
                nc.tensor.transpose(
                    pst[0:128, 0:32],
                    s32_sb[:, 128 * s : 128 * (s + 1)],
                    ident_sb[0:32, 0:32],
                )
                negmx = smp.tile([128, 1], F32, tag="negmx")
                nc.vector.reduce_max(
                    negmx[:], pst[0:128, 0:32], axis=mybir.AxisListType.X, negate=True
                )
                wraw = smp.tile([128, 32], F32, tag="wraw")
                nc.scalar.activation(wraw[:], pst[0:128, 0:32], ACTF.Exp, bias=negmx[:])
                zsum = smp.tile([128, 1], F32, tag="zsum")
                nc.vector.reduce_sum(zsum[:], wraw[:], axis=mybir.AxisListType.X)
                rz = smp.tile([128, 1], F32, tag="rz")
                nc.vector.reciprocal(rz[:], zsum[:])
                w_b = smp.tile([128, 32], F32, tag="wb")
                nc.vector.tensor_scalar_mul(w_b[:], wraw[:], rz[:])
                wsum = smp.tile([128, 1], F32, tag="wsum")
                nc.vector.reduce_sum(wsum[:], w_b[:], axis=mybir.AxisListType.X)
                nc.vector.tensor_add(racc[:], racc[:], wsum[:])
                pwt = pT.tile([128, 128], F32, tag="pt")
                nc.tensor.transpose(pwt[0:32, 0:128], w_b[:], ident_sb[:])
                nc.vector.tensor_copy(wT[:, 128 * s : 128 * (s + 1)], pwt[0:32, 0:128])

            # ---- pass 2: broadcast w (PE fp16), scale + accumulate (DVE) ----
            yT_sb = yap.tile([64, F], F32, tag="yT")
            for e in range(E):
                wrow = smp.tile([1, F], F16, tag="wrow")
                nc.sync.dma_start(wrow[:], wT[e : e + 1, :])
                pb = pB.tile([64, F], F32, tag="pb")
                nc.tensor.matmul(pb[:], ones64_sb[:], wrow[:], start=True, stop=True)
                sv = hp.tile([64, F], F32, tag="sv")
                nc.vector.tensor_mul(sv[:], evs[:, e, :], pb[:])
                if e == 0:
                    nc.vector.tensor_copy(yT_sb[:], sv[:])
                else:
                    nc.vector.tensor_add(yT_sb[:], yT_sb[:], sv[:])
            ytiles.append((f, yT_sb))

        while ytiles:
            emit_tail()

        # ---- reg-loss partial: sum racc over partitions ----
        prs = pT.tile([128, 128], F32, tag="pt")
        nc.tensor.matmul(prs[0:1, 0:1], onescol_sb[:], racc[:], start=True, stop=True)
        rs_sb = outp.tile([1, 1], F32, tag="rssb")
        nc.vector.tensor_copy(rs_sb[:], prs[0:1, 0:1])
        nc.sync.dma_start(rs_d, rs_sb[:])

    nc.compile()
    return nc


_NC_CACHE = {}


def _get_program():
    key = MATMUL_DTYPE
    if key not in _NC_CACHE:
        _NC_CACHE[key] = _build_program()
    return _NC_CACHE[key]


def kernel(**inputs):
    x = np.asarray(inputs["backbone_output"], dtype=np.float32)
    task = np.asarray(inputs["task"]).astype(np.int64)
    W1 = np.asarray(inputs["W1"], dtype=np.float64)
    b1 = np.asarray(inputs["b1"], dtype=np.float32)
    W2 = np.asarray(inputs["W2"], dtype=np.float64)
    b2 = np.asarray(inputs["b2"], dtype=np.float64)
    TQ = np.asarray(inputs["task_queries"], dtype=np.float64)
    KM = np.asarray(inputs["key_matricies"], dtype=np.float64)
    VM = np.asarray(inputs["value_matricies"], dtype=np.float64)

    # ---- exact algebraic folding (float64) ----
    W2K = np.einsum("eho,eoj->ehj", W2, KM)          # [E,256,64]
    W2V = np.einsum("eho,eoj->ehj", W2, VM)
    G = np.einsum("ehj,tj->eht", W2K, TQ)            # [E,256,16]
    cET = np.einsum("eo,eoj,tj->et", b2, KM, TQ)     # [E,16]
    b2V = np.einsum("eo,eoj->ej", b2, VM)            # [E,64]

    # augmented VS weights: cols 0:64 values, cols 64+32q:80+32q scores
    waug = np.zeros((E, D_IN, 128), np.float64)
    waug[:, :, 0:64] = W2V
    for e in range(E):
        q = e % 2
        waug[e, :, 64 + 32 * q : 80 + 32 * q] = G[e]
    waug = np.ascontiguousarray(
        waug.reshape(E, 2, 128, 128).transpose(2, 0, 1, 3)
    ).astype(np.float32)                             # [128,E,2,128]
    w1r = np.ascontiguousarray(
        np.asarray(W1, np.float32).reshape(E, 2, 128, H1).transpose(2, 0, 1, 3)
    )                                                # [128,E,2,H1]
    b1r = np.ascontiguousarray(b1.reshape(E, 2, 128).transpose(2, 1, 0))  # [128,2,E]
    b2aug = np.zeros((128, E), np.float32)
    b2aug[0:64, :] = b2V.T
    for e in range(E):
        q = e % 2
        b2aug[64 + 32 * q : 80 + 32 * q, e] = cET[e]

    ut = np.zeros((T, B), np.float32)
    ut[task, np.arange(B)] = 1.0
    ut4 = np.zeros((64, B), np.float32)   # rows 0:16, 32:48 = onehot; rest 0
    ut4[0:16] = ut
    ut4[32:48] = ut

    zc4 = np.zeros((128, 62), np.float32)
    for q in range(2):
        zc4[64 + 32 * q : 80 + 32 * q, 30 + q] = 1.0
    ones64 = np.ones((1, 64), np.float16)
    ident = np.eye(128, dtype=np.float32)
    onescol = np.ones((128, 1), np.float32)

    shared = {
        "w1": w1r,
        "waug": waug,
        "b1r": b1r,
        "b2aug": b2aug,
        "zc4": zc4,
        "ones64": ones64,
        "ident": ident,
        "onescol": onescol,
    }
    xt = np.ascontiguousarray(x.T.reshape(2, 128, B))   # [2,128,B]
    in_maps = []
    for c in range(NCORES):
        sl = slice(c * BC, (c + 1) * BC)
        in_maps.append({"xt": xt[:, :, sl], "ut4": ut4[:, sl], **shared})

    nc = _get_program()
    trace = os.environ.get("MOE_TRACE", "0") == "1"
    br = run_bass_kernel_spmd(nc, in_maps, list(range(NCORES)), trace=trace)
    res = br.results
    global LAST_EXEC_NS
    LAST_EXEC_NS = br.exec_time_ns
    if trace and br.instructions_and_trace is not None:
        print("trace:", br.instructions_and_trace[1])

    y = np.concatenate([res[c]["y"] for c in range(NCORES)], axis=0)
    wsum = float(sum(res[c]["rsum"][0, 0] for c in range(NCORES)))
    reg = np.float32(-(MU / E) * (wsum + EPS * B * E))
    return y, reg
